# revision 1
# baseline (speedup 1.0000x reference)
"""MoE top-1 routing kernel for Trainium2 (8 NeuronCores, expert-parallel).

Strategy:
  - Gate (x @ Wg.T + bg, argmax) is computed on host in float64. The min
    top-2 logit gap for this problem's data is ~1.2e-5, orders of magnitude
    above any fp32 backend's rounding noise (~1e-6), so the fp64 argmax
    matches the fp32 reference argmax exactly.
  - Tokens are grouped by expert on host (the "all-to-all dispatch");
    core e receives expert e's tokens (capacity-padded) plus expert e's
    weights, and runs the dense SwiGLU FFN for just those tokens.
  - Outputs are scattered back to token order on host (the "combine").
    With top-1 routing the combine weight is exactly 1.0.

Device kernel (per core), all matmuls on the PE array:
  h1^T = W1 x^T   (contract D, f on partitions)
  h2^T = W2 x^T
  g^T  = silu(h1^T) * h2^T
  y^T  = W3 g^T    (contract F, d on partitions)
All tensors are staged transposed (feature-major) so the PE contraction
dim always sits on partitions; the host does the transposes.
"""

import sys
from contextlib import ExitStack

if "/opt/trn_rl_repo" not in sys.path:
    sys.path.insert(0, "/opt/trn_rl_repo")

import numpy as np

P = 128
D = 768          # model dim
E = 8            # experts == cores
F = 469          # ffn hidden
FP = 512         # F padded to a multiple of 128
KT = D // P      # 6 k-tiles over D
MT = FP // P     # 4 f-tiles over padded F
DT = D // P      # 6 out-tiles over D
MIN_C = 128                # capacity floor; actual C adapts to max expert load
CHUNK = 512                # moving-operand free dim per matmul

# "float32" | "float32r" | "bfloat16" — matmul input precision on device.
MM_MODE = "float32r"

# pool buffer counts (tunable)
BUFS = {"x": 3, "g": 2, "s": 4, "o": 6, "ps": 8}
CHUNK_SIZES = None   # explicit chunk-size list override (else balanced split)
A_GROUP = 2          # f-tiles accumulated concurrently in stage A (1, 2, or 4)
B_SPLIT = False      # start stage-B early during pair 1 (hurts: PSUM pressure)
WARMUP_MMS = 0      # dummy matmuls during the DMA preload to pre-warm the PE clock
W3_HALVES = True     # load w3 in two d-halves so stage B starts sooner
# DMA plumbing knobs
X_MERGE = False       # one merged x DMA per chunk (vs 6 per-k DMAs)
W_MERGE = False       # single DMA each for W2/W3 (vs per-k/-m)
STORE_GPSIMD = False  # stores via SWDGE/Pool (vs HWDGE/sync)

_cache = {}


def _np_in_dtype():
    if MM_MODE == "bfloat16":
        import ml_dtypes

        return np.dtype(ml_dtypes.bfloat16)
    return np.dtype(np.float32)


def _build(C):
    """Build + compile the per-core Tile kernel for capacity C tokens."""
    import concourse.bacc as bacc
    import concourse.tile as tile
    from concourse import mybir

    f32 = mybir.dt.float32
    # float32r = fp32 bytes, reduced-precision PE multiply (full matmul rate
    # at >=256 moving columns vs fp32's 1/4 rate; rel-err ~2e-4 on this net).
    # Declared natively so the BIR verifier sees f32r producers end-to-end.
    in_dt = {
        "bfloat16": mybir.dt.bfloat16,
        "float32r": mybir.dt.float32r,
        "float32": mybir.dt.float32,
    }[MM_MODE]

    def mm_view(ap):
        return ap

    nc = bacc.Bacc("TRN2", target_bir_lowering=False, debug=False, num_devices=E)

    xt = nc.dram_tensor("xt", [KT, P, C], in_dt, kind="ExternalInput").ap()
    w1t = nc.dram_tensor("w1t", [KT, P, FP], in_dt, kind="ExternalInput").ap()
    w2t = nc.dram_tensor("w2t", [KT, P, FP], in_dt, kind="ExternalInput").ap()
    w3t = nc.dram_tensor("w3t", [MT, P, D], in_dt, kind="ExternalInput").ap()
    yt = nc.dram_tensor("yt", [DT, P, C], f32, kind="ExternalOutput").ap()

    # Balanced chunk split: f32r matmuls drop to 1/4 rate below 256 columns,
    # so keep every chunk >= 256 (when C allows) instead of a ragged 512-tail.
    # Chunk sizes must be EVEN: odd moving-column counts fail the walrus
    # is_valid_s3d3_mm ISA check for 4-byte matmul dtypes.
    assert C % 2 == 0, C
    if CHUNK_SIZES is not None:
        sizes = list(CHUNK_SIZES)
    else:
        nch = max(1, -(-C // CHUNK))
        u, uextra = divmod(C // 2, nch)
        sizes = [2 * (u + (1 if i < uextra else 0)) for i in range(nch)]
    chunks = []
    off = 0
    for nn in sizes:
        chunks.append((off, nn))
        off += nn
    assert off == C, (off, C)

    silu = mybir.ActivationFunctionType.Silu

    with tile.TileContext(nc) as tc, ExitStack() as ctx:
        wpool = ctx.enter_context(tc.tile_pool(name="w", bufs=1))
        xpool = ctx.enter_context(tc.tile_pool(name="x", bufs=BUFS["x"]))
        gpool = ctx.enter_context(tc.tile_pool(name="g", bufs=BUFS["g"]))
        spool = ctx.enter_context(tc.tile_pool(name="s", bufs=BUFS["s"]))
        opool = ctx.enter_context(tc.tile_pool(name="o", bufs=BUFS["o"]))
        # one shared PSUM tag: 8 slots = all 8 banks; stage A holds up to 8
        # accumulators (h1 x4, h2 x4), stage B grabs slots as they free
        pspool = ctx.enter_context(
            tc.tile_pool(name="ps", bufs=BUFS["ps"], space="PSUM")
        )

        w1_sb = [
            wpool.tile([P, FP], in_dt, tag=f"w1_{k}", name=f"w1_{k}")
            for k in range(KT)
        ]

        def w1s(k, m):
            return w1_sb[k][:, m * P : (m + 1) * P]

        if W_MERGE:
            w2_sb = wpool.tile([P, KT, FP], in_dt, tag="w2", name="w2_sb")
            w3_sb = wpool.tile([P, MT, D], in_dt, tag="w3", name="w3_sb")
            w2s = lambda k: w2_sb[:, k, :]
            w3s = lambda m: w3_sb[:, m, :]
        else:
            w2_l = [
                wpool.tile([P, FP], in_dt, tag=f"w2_{k}", name=f"w2_{k}")
                for k in range(KT)
            ]
            w3_l = [
                wpool.tile([P, D], in_dt, tag=f"w3_{m}", name=f"w3_{m}")
                for m in range(MT)
            ]
            w2s = lambda k: w2_l[k][:]
            w3s = lambda m: w3_l[m][:]

        # dummy matmuls on a zeroed tile fill the DMA-preload window so the
        # PE clock ramp (cold 1.2GHz -> warm 2.4GHz after ~3us sustained) is
        # already paid before the first real matmul; outputs are never used
        if WARMUP_MMS:
            warm = wpool.tile([P, 256], in_dt, tag="warm", name="warm")
            nc.vector.memset(warm[:], 0.0)
            wps = pspool.tile([P, 256], f32, tag="ps", name="wps")
            for _ in range(WARMUP_MMS):
                nc.tensor.matmul(wps[:], warm[:, :P], warm[:], start=True, stop=True)
            wsink = wpool.tile([P, 256], f32, tag="wsink", name="wsink")
            nc.scalar.copy(wsink[:], wps[:])  # consume so the PSUM slot frees

        # chunk-0 x interleaved with W1 (both split per k) so the k-outer pass
        # starts after the first ~0.5MB of DMA instead of the full preload
        nn0 = chunks[0][1]
        x0 = [
            xpool.tile([P, nn0], in_dt, tag=f"x0_{k}", name=f"x0_{k}")
            for k in range(KT)
        ]
        for k in range(KT):
            nc.sync.dma_start(w1_sb[k][:], w1t[k])
            nc.sync.dma_start(x0[k][:], xt[k, :, 0:nn0])
        if W_MERGE:
            nc.sync.dma_start(w2_sb[:], w2t.rearrange("k p f -> p k f"))
            nc.sync.dma_start(w3_sb[:], w3t.rearrange("m p d -> p m d"))
        else:
            for k in range(KT):
                nc.sync.dma_start(w2s(k), w2t[k])
            for m in range(MT):
                if W3_HALVES:
                    nc.sync.dma_start(w3s(m)[:, : D // 2], w3t[m, :, : D // 2])
                    nc.sync.dma_start(w3s(m)[:, D // 2 :], w3t[m, :, D // 2 :])
                else:
                    nc.sync.dma_start(w3s(m), w3t[m])

        for ci, (n0, nn) in enumerate(chunks):
            if ci == 0:
                xn = x0
            elif X_MERGE:
                xnt = xpool.tile([P, KT, nn], in_dt, tag="xn", name="xnt")
                nc.sync.dma_start(
                    xnt[:], xt[:, :, n0 : n0 + nn].rearrange("k p n -> p k n")
                )
                xn = [xnt[:, k, :] for k in range(KT)]
            else:
                xn = [
                    xpool.tile([P, nn], in_dt, tag=f"xn_{k}", name=f"xn_{k}")
                    for k in range(KT)
                ]
                for k in range(KT):
                    nc.sync.dma_start(xn[k][:], xt[k, :, n0 : n0 + nn])

            # stage A in m-pairs, k-outer inside: only 4 PSUM banks held at
            # a time (vs 8), leaving headroom for stage-B/next-chunk overlap;
            # the k-outer inner order still lets chunk-0 start after the
            # first w1/x k-tile lands
            # number of d-tiles whose stage-B (m=0,1) matmuls are emitted
            # early, between stage-A pairs; capped at 2 so PSUM stays at
            # 2 (open pso) + 4 (pair-1 accumulators) + slack <= 8 banks
            early_d = 2 if (B_SPLIT and A_GROUP == 2 and MT == 4) else 0
            psos = {}
            gs = []
            for mp in range(MT // A_GROUP):
                ms = tuple(range(A_GROUP * mp, A_GROUP * (mp + 1)))
                ps1 = {
                    m: pspool.tile([P, nn], f32, tag="ps", name=f"ps1_{m}")
                    for m in ms
                }
                for k in range(KT):
                    for m in ms:
                        nc.tensor.matmul(
                            ps1[m][:],
                            mm_view(w1s(k, m)),
                            mm_view(xn[k][:]),
                            start=(k == 0),
                            stop=(k == KT - 1),
                        )
                ps2 = {
                    m: pspool.tile([P, nn], f32, tag="ps", name=f"ps2_{m}")
                    for m in ms
                }
                for k in range(KT):
                    for m in ms:
                        nc.tensor.matmul(
                            ps2[m][:],
                            mm_view(w2s(k)[:, m * P : (m + 1) * P]),
                            mm_view(xn[k][:]),
                            start=(k == 0),
                            stop=(k == KT - 1),
                        )
                for m in ms:
                    sil = spool.tile([P, nn], f32, tag="sil", name="sil")
                    nc.scalar.activation(sil[:], ps1[m][:], silu)
                    g = gpool.tile([P, nn], in_dt, tag=f"g{m}", name=f"g{m}")
                    nc.vector.tensor_mul(g[:], sil[:], ps2[m][:])
                    gs.append(g)
                if mp == 0:
                    # overlap: open the first stage-B accumulators using the
                    # already-finished g0/g1 while pair 1 is still on the PE
                    for d in range(early_d):
                        pso = pspool.tile([P, nn], f32, tag="ps", name="pso")
                        psos[d] = pso
                        for m in ms:
                            nc.tensor.matmul(
                                pso[:],
                                mm_view(w3s(m)[:, d * P : (d + 1) * P]),
                                mm_view(gs[m][:]),
                                start=(m == 0),
                                stop=False,
                            )

            for d in range(DT):
                if d in psos:
                    pso = psos[d]
                    rest = range(A_GROUP, MT)
                else:
                    pso = pspool.tile([P, nn], f32, tag="ps", name="pso")
                    rest = range(MT)
                for m in rest:
                    nc.tensor.matmul(
                        pso[:],
                        mm_view(w3s(m)[:, d * P : (d + 1) * P]),
                        mm_view(gs[m][:]),
                        start=(m == 0),
                        stop=(m == MT - 1),
                    )
                ot = opool.tile([P, nn], f32, tag="ot", name="ot")
                # alternate copy engine so the stage-B epilogue isn't
                # serialized on ACT alone
                if d % 2 == 1:
                    nc.scalar.copy(ot[:], pso[:])
                else:
                    nc.vector.tensor_copy(ot[:], pso[:])
                eng = nc.gpsimd if STORE_GPSIMD else nc.sync
                eng.dma_start(yt[d, :, n0 : n0 + nn], ot[:])

    nc.compile()
    return nc


LAST_RESULTS = None  # BassKernelResults of the most recent run (for test harness)


def kernel(x, Wg, bg, W1, W2, W3):
    global LAST_RESULTS
    from concourse.bass_utils import run_bass_kernel_spmd

    x = np.asarray(x)
    Wg, bg = np.asarray(Wg), np.asarray(bg)
    W1, W2, W3 = np.asarray(W1), np.asarray(W2), np.asarray(W3)
    B, S, d = x.shape
    T = B * S
    assert d == D and Wg.shape == (E, D)

    xf = np.ascontiguousarray(x.reshape(T, D))

    # ---- host gate + top-1 routing (fp64: exact vs any fp32 backend) ----
    gate = xf.astype(np.float64) @ Wg.astype(np.float64).T + bg.astype(np.float64)
    eid = np.argmax(gate, axis=1)
    counts = np.bincount(eid, minlength=E)
    order = np.argsort(eid, kind="stable")
    offs = np.concatenate(([0], np.cumsum(counts)))

    C = max(MIN_C, 2 * int(-(-counts.max() // 2)))
    key = (C, MM_MODE)
    if key not in _cache:
        _cache[key] = _build(C)
    nc = _cache[key]

    in_dt = _np_in_dtype()

    # ---- build per-core inputs (dispatch) ----
    in_maps = []
    tok_lists = []
    for e in range(E):
        toks = order[offs[e] : offs[e + 1]]
        tok_lists.append(toks)
        ce = len(toks)
        xeT = np.zeros((D, C), dtype=in_dt)
        if ce:
            xeT[:, :ce] = xf[toks].T.astype(in_dt)
        w1 = np.zeros((D, FP), dtype=in_dt)
        w1[:, :F] = W1[e].T.astype(in_dt)
        w2 = np.zeros((D, FP), dtype=in_dt)
        w2[:, :F] = W2[e].T.astype(in_dt)
        w3 = np.zeros((FP, D), dtype=in_dt)
        w3[:F, :] = W3[e].T.astype(in_dt)
        in_maps.append(
            {
                "xt": np.ascontiguousarray(xeT.reshape(KT, P, C)),
                "w1t": np.ascontiguousarray(w1.reshape(KT, P, FP)),
                "w2t": np.ascontiguousarray(w2.reshape(KT, P, FP)),
                "w3t": np.ascontiguousarray(w3.reshape(MT, P, D)),
            }
        )

    res = run_bass_kernel_spmd(nc, in_maps, list(range(E)))
    LAST_RESULTS = res

    # ---- combine: scatter outputs back to token order ----
    y = np.empty((T, D), dtype=np.float32)
    for e in range(E):
        toks = tok_lists[e]
        if len(toks):
            yte = res.results[e]["yt"].reshape(D, C)
            y[toks] = yte[:, : len(toks)].T
    return y.reshape(B, S, d)



# revision 6
# speedup vs baseline: 1.2117x; 1.2117x over previous
"""MoE top-1 routing kernel for Trainium2 (8 NeuronCores, expert-parallel).

Strategy:
  - Gate (x @ Wg.T + bg, argmax) on host in float64 (exact vs fp32 ref:
    min top-2 gap ~1.2e-5 >> fp32 rounding noise).
  - Tokens grouped by expert on host (the all-to-all dispatch); core e runs
    the dense SwiGLU FFN for expert e's tokens (capacity-padded to C).
  - Combine on host (top-1 => weight 1.0); outputs come back bf16 and are
    upcast to f32.

Device kernel: all matmuls are fp8(e4m3) DoubleRow (2 k-tiles of 128 per
matmul, 0.5 PE cycles/output-row — 2x the bf16/f32r rate). Full fp8 would
lose ~5% accuracy, so every operand is hi/lo error-compensated:

  A = fp8(s*A) [hi] + fp8(16*(s*A - hi))/16 [lo],  residual ~0.16%

and each 128-contraction product uses 3 DoubleRow terms:
  W*x ~= Whi*xhi + Whi*xlo_s + Wlo*xhi_s          (stage A, h1/h2)
  W3*g ~= W3hi*ghi + W3hi*glo + W3lo*ghi_s        (stage B)
where *_s = fp8(hi/16) (exact exponent shift), xlo_s = fp8(xlo/16).
End-to-end rel-l2 vs the f32 reference: ~2.5e-3.

Scales (powers of 2, exact): x*8, W1*128, W2*4, W3*128. The products give
ps1 = 1024*h1 (silu applied with input-scale 2^-10), ps2 = 32*h2,
p = sil*ps2 = 32*g (|p|max ~160 < fp8 max 240), out accum = 4096*y.

PE cost: 54 cycles/token (36 stage A + 18 stage B) vs 72 for bf16/f32r.
Per-chunk schedule is software-pipelined: chunk c's stage-B halves are
emitted between chunk c+1's stage-A pairs so the PE never waits for the
ACT/DVE epilogue chain (silu -> p -> ghi -> glo/ghi_s).
"""

import sys
from contextlib import ExitStack

if "/opt/trn_rl_repo" not in sys.path:
    sys.path.insert(0, "/opt/trn_rl_repo")

import numpy as np

P = 128
D = 768          # model dim
E = 8            # experts == cores
F = 469          # ffn hidden
FP = 512         # F padded to a multiple of 128
KP = 3           # k-PAIRS over D (6 k-tiles of 128)
MT = 4           # f-tiles over padded F
FPAIR = 2        # f-pairs for stage-B contraction
DT = D // P      # 6 out-tiles over D
MIN_C = 128      # capacity floor
CHUNK = 512      # tokens per chunk (PSUM bank = 512 f32)

SX = 8.0         # x quant scale (2^3)
SW1 = 128.0      # W1 quant scale (2^7)
SW2 = 4.0        # W2 quant scale (2^2): p = 32*g stays under fp8 max 240
SW3 = 128.0      # W3 quant scale (2^7)
SILU_SCALE = 1.0 / (SX * SW1)        # 2^-10
OUT_SCALE = 1.0 / (SW3 * SX * SW2)   # 2^-12

_cache = {}


def _chunks(C):
    """[512]*k + small even tail (small tail keeps the pipeline drain short)."""
    sizes = []
    left = C
    while left > CHUNK:
        sizes.append(CHUNK)
        left -= CHUNK
    sizes.append(left)
    assert all(s % 2 == 0 for s in sizes) and sum(sizes) == C
    out = []
    off = 0
    for s in sizes:
        out.append((off, s))
        off += s
    return out


def _build(C):
    import concourse.bacc as bacc
    import concourse.tile as tile
    from concourse import mybir

    f32 = mybir.dt.float32
    f8 = mybir.dt.float8e4
    bf16 = mybir.dt.bfloat16
    DR = mybir.MatmulPerfMode.DoubleRow
    silu = mybir.ActivationFunctionType.Silu

    nc = bacc.Bacc("TRN2", target_bir_lowering=False, debug=False, num_devices=E)

    # DRAM inputs, all pre-packed on host as [KP, P, 2, cols] fp8
    xh_d = nc.dram_tensor("xhi", [KP, P, 2, C], f8, kind="ExternalInput").ap()
    xls_d = nc.dram_tensor("xlos", [KP, P, 2, C], f8, kind="ExternalInput").ap()
    xhs_d = nc.dram_tensor("xhis", [KP, P, 2, C], f8, kind="ExternalInput").ap()
    w1h_d = nc.dram_tensor("w1hi", [KP, P, 2, FP], f8, kind="ExternalInput").ap()
    w1l_d = nc.dram_tensor("w1lo", [KP, P, 2, FP], f8, kind="ExternalInput").ap()
    w2h_d = nc.dram_tensor("w2hi", [KP, P, 2, FP], f8, kind="ExternalInput").ap()
    w2l_d = nc.dram_tensor("w2lo", [KP, P, 2, FP], f8, kind="ExternalInput").ap()
    w3h_d = nc.dram_tensor("w3hi", [FPAIR, P, 2, D], f8, kind="ExternalInput").ap()
    w3l_d = nc.dram_tensor("w3lo", [FPAIR, P, 2, D], f8, kind="ExternalInput").ap()
    yt_d = nc.dram_tensor("yt", [DT, P, C], bf16, kind="ExternalOutput").ap()

    chunks = _chunks(C)
    nch = len(chunks)
    Chalf = (C // 2 + 1) // 2 * 2  # even split point for the 2-phase x preload

    with tile.TileContext(nc) as tc, ExitStack() as ctx:
        wpool = ctx.enter_context(tc.tile_pool(name="w", bufs=1))
        spool = ctx.enter_context(tc.tile_pool(name="s", bufs=3))
        ppool = ctx.enter_context(tc.tile_pool(name="p", bufs=3))
        gpool = ctx.enter_context(tc.tile_pool(name="g", bufs=2))
        opool = ctx.enter_context(tc.tile_pool(name="o", bufs=4))
        pspool = ctx.enter_context(tc.tile_pool(name="ps", bufs=8, space="PSUM"))

        xh = [wpool.tile([P, 2, C], f8, tag=f"xh{k}", name=f"xh{k}") for k in range(KP)]
        xls = [wpool.tile([P, 2, C], f8, tag=f"xls{k}", name=f"xls{k}") for k in range(KP)]
        xhs = [wpool.tile([P, 2, C], f8, tag=f"xhs{k}", name=f"xhs{k}") for k in range(KP)]
        w1h = [wpool.tile([P, 2, FP], f8, tag=f"w1h{k}", name=f"w1h{k}") for k in range(KP)]
        w1l = [wpool.tile([P, 2, FP], f8, tag=f"w1l{k}", name=f"w1l{k}") for k in range(KP)]
        w2h = [wpool.tile([P, 2, FP], f8, tag=f"w2h{k}", name=f"w2h{k}") for k in range(KP)]
        w2l = [wpool.tile([P, 2, FP], f8, tag=f"w2l{k}", name=f"w2l{k}") for k in range(KP)]
        w3h = [wpool.tile([P, 2, D], f8, tag=f"w3h{k}", name=f"w3h{k}") for k in range(FPAIR)]
        w3l = [wpool.tile([P, 2, D], f8, tag=f"w3l{k}", name=f"w3l{k}") for k in range(FPAIR)]

        # ---- preload, ordered to match first-chunk consumption order ----
        # phase 1: hi weights + first-half x (covers chunks in [0, Chalf))
        for kp in range(KP):
            nc.sync.dma_start(w1h[kp][:], w1h_d[kp])
            nc.sync.dma_start(xh[kp][:, :, :Chalf], xh_d[kp, :, :, :Chalf])
            nc.sync.dma_start(w2h[kp][:], w2h_d[kp])
        for kp in range(KP):
            nc.sync.dma_start(xls[kp][:, :, :Chalf], xls_d[kp, :, :, :Chalf])
        for kp in range(KP):
            nc.sync.dma_start(w1l[kp][:], w1l_d[kp])
            nc.sync.dma_start(xhs[kp][:, :, :Chalf], xhs_d[kp, :, :, :Chalf])
            nc.sync.dma_start(w2l[kp][:], w2l_d[kp])
        for fp2 in range(FPAIR):
            nc.sync.dma_start(w3h[fp2][:], w3h_d[fp2])
            nc.sync.dma_start(w3l[fp2][:], w3l_d[fp2])
        # phase 2: second-half x
        if Chalf < C:
            for kp in range(KP):
                nc.sync.dma_start(xh[kp][:, :, Chalf:], xh_d[kp, :, :, Chalf:])
            for kp in range(KP):
                nc.sync.dma_start(xls[kp][:, :, Chalf:], xls_d[kp, :, :, Chalf:])
            for kp in range(KP):
                nc.sync.dma_start(xhs[kp][:, :, Chalf:], xhs_d[kp, :, :, Chalf:])

        def msl(m):
            return slice(m * P, (m + 1) * P)

        def stage_a_pair(ci, mp):
            """ps1/ps2 accumulators for f-tiles (2mp, 2mp+1). 9 DR matmuls each."""
            n0, nn = chunks[ci]
            csl = slice(n0, n0 + nn)
            ms = (2 * mp, 2 * mp + 1)
            ps1 = {m: pspool.tile([P, nn], f32, tag="ps", name=f"ps1_{ci}_{m}") for m in ms}
            ps2 = {m: pspool.tile([P, nn], f32, tag="ps", name=f"ps2_{ci}_{m}") for m in ms}
            for wh, wl, psd in ((w1h, w1l, ps1), (w2h, w2l, ps2)):
                for kp in range(KP):
                    for m in ms:
                        nc.tensor.matmul(psd[m][:], wh[kp][:, :, msl(m)],
                                         xh[kp][:, :, csl], start=(kp == 0),
                                         stop=False, perf_mode=DR)
                for kp in range(KP):
                    for m in ms:
                        nc.tensor.matmul(psd[m][:], wh[kp][:, :, msl(m)],
                                         xls[kp][:, :, csl], start=False,
                                         stop=False, perf_mode=DR)
                for kp in range(KP):
                    for m in ms:
                        nc.tensor.matmul(psd[m][:], wl[kp][:, :, msl(m)],
                                         xhs[kp][:, :, csl], start=False,
                                         stop=(kp == KP - 1), perf_mode=DR)
            return ps1, ps2

        def epilogue_pair(ci, mp, ps1, ps2, g):
            """silu -> p -> {ghi, glo, ghi_s} for the two f-tiles of pair mp."""
            n0, nn = chunks[ci]
            gh, gl, gs = g[mp]
            for j, m in enumerate((2 * mp, 2 * mp + 1)):
                sil = spool.tile([P, nn], f32, tag="sil", name=f"sil{ci}_{m}")
                nc.scalar.activation(sil[:], ps1[m][:], silu, scale=SILU_SCALE)
                p = ppool.tile([P, nn], f32, tag="p", name=f"p{ci}_{m}")
                nc.vector.tensor_mul(p[:], sil[:], ps2[m][:])
                nc.scalar.copy(gh[:, j, :], p[:])            # ghi = fp8(p)
                nc.vector.tensor_sub(gl[:, j, :], p[:], gh[:, j, :])   # glo
                nc.vector.tensor_scalar_mul(gs[:, j, :], gh[:, j, :], 1.0 / 16.0)

        def stage_b_half(ci, half, g, ots):
            """d-tiles [3*half, 3*half+3): 6 DR matmuls each + scaled copy + store."""
            n0, nn = chunks[ci]
            for d in range(3 * half, 3 * half + 3):
                pso = pspool.tile([P, nn], f32, tag="ps", name=f"pso{ci}_{d}")
                for fp2 in range(FPAIR):
                    gh, gl, gs = g[fp2]
                    nc.tensor.matmul(pso[:], w3h[fp2][:, :, msl(d)], gh[:],
                                     start=(fp2 == 0), stop=False, perf_mode=DR)
                    nc.tensor.matmul(pso[:], w3h[fp2][:, :, msl(d)], gl[:],
                                     start=False, stop=False, perf_mode=DR)
                    nc.tensor.matmul(pso[:], w3l[fp2][:, :, msl(d)], gs[:],
                                     start=False, stop=(fp2 == FPAIR - 1),
                                     perf_mode=DR)
                q, r = divmod(d, 2)
                if r == 0:
                    ot = opool.tile([P, 2, nn], bf16, tag="ot", name=f"ot{ci}_{q}")
                    ots[q] = ot
                    nc.scalar.mul(ot[:, 0, :], pso[:], OUT_SCALE)
                else:
                    ot = ots[q]  # slot 0 was filled by d-1 (possibly in half 0)
                    nc.vector.tensor_scalar_mul(ot[:, 1, :], pso[:], OUT_SCALE)
                    nc.sync.dma_start(
                        yt_d[2 * q:2 * q + 2, :, n0:n0 + nn].rearrange("j p c -> p j c"),
                        ot[:],
                    )

        # ---- software-pipelined emission ----
        prev = None  # (ci, g, ots) of the chunk whose stage B is pending
        for ci in range(nch):
            g = {mp: (gpool.tile([P, 2, chunks[ci][1]], f8, tag=f"gh{mp}", name=f"gh{ci}_{mp}"),
                      gpool.tile([P, 2, chunks[ci][1]], f8, tag=f"gl{mp}", name=f"gl{ci}_{mp}"),
                      gpool.tile([P, 2, chunks[ci][1]], f8, tag=f"gs{mp}", name=f"gs{ci}_{mp}"))
                 for mp in range(2)}
            ps1a, ps2a = stage_a_pair(ci, 0)
            if prev is not None:
                stage_b_half(prev[0], 0, prev[1], prev[2])
            epilogue_pair(ci, 0, ps1a, ps2a, g)
            ps1b, ps2b = stage_a_pair(ci, 1)
            if prev is not None:
                stage_b_half(prev[0], 1, prev[1], prev[2])
            epilogue_pair(ci, 1, ps1b, ps2b, g)
            prev = (ci, g, {})
        stage_b_half(prev[0], 0, prev[1], prev[2])
        stage_b_half(prev[0], 1, prev[1], prev[2])

    nc.compile()
    return nc


LAST_RESULTS = None  # BassKernelResults of the most recent run (for test harness)


def kernel(x, Wg, bg, W1, W2, W3):
    global LAST_RESULTS
    import ml_dtypes
    from concourse.bass_utils import run_bass_kernel_spmd

    f8np = ml_dtypes.float8_e4m3

    x = np.asarray(x)
    Wg, bg = np.asarray(Wg), np.asarray(bg)
    W1, W2, W3 = np.asarray(W1), np.asarray(W2), np.asarray(W3)
    B, S, d = x.shape
    T = B * S
    assert d == D and Wg.shape == (E, D)

    xf = np.ascontiguousarray(x.reshape(T, D))

    # ---- host gate + top-1 routing (fp64: exact vs any fp32 backend) ----
    gate = xf.astype(np.float64) @ Wg.astype(np.float64).T + bg.astype(np.float64)
    eid = np.argmax(gate, axis=1)
    counts = np.bincount(eid, minlength=E)
    order = np.argsort(eid, kind="stable")
    offs = np.concatenate(([0], np.cumsum(counts)))

    C = max(MIN_C, 2 * int(-(-counts.max() // 2)))
    if C not in _cache:
        _cache[C] = _build(C)
    nc = _cache[C]

    def q8(a):
        return a.astype(f8np)

    def hi_lo(a, s):
        hi = q8(a * s)
        lo16 = q8((a * s - hi.astype(np.float32)) * 16.0)
        return hi, lo16

    def pack(a, npair):
        # [R, cols] with R = npair*2*128 -> [npair, P, 2, cols]
        cols = a.shape[1]
        return np.ascontiguousarray(
            a.reshape(npair, 2, P, cols).transpose(0, 2, 1, 3)
        )

    # ---- per-core inputs (dispatch) ----
    in_maps = []
    tok_lists = []
    for e in range(E):
        toks = order[offs[e]:offs[e + 1]]
        tok_lists.append(toks)
        ce = len(toks)

        xT = np.zeros((D, C), dtype=np.float32)
        if ce:
            xT[:, :ce] = xf[toks].T
        xhi, xlo = hi_lo(xT, SX)
        xhif = xhi.astype(np.float32)
        xlos = q8(xlo.astype(np.float32) / 16.0)
        xhis = q8(xhif / 16.0)

        w1T = np.zeros((D, FP), dtype=np.float32)
        w1T[:, :F] = W1[e].T
        w1hi, w1lo = hi_lo(w1T, SW1)
        w2T = np.zeros((D, FP), dtype=np.float32)
        w2T[:, :F] = W2[e].T
        w2hi, w2lo = hi_lo(w2T, SW2)
        w3T = np.zeros((FP, D), dtype=np.float32)
        w3T[:F, :] = W3[e].T
        w3hi, w3lo = hi_lo(w3T, SW3)

        in_maps.append({
            "xhi": pack(xhi, KP),
            "xlos": pack(xlos, KP),
            "xhis": pack(xhis, KP),
            "w1hi": pack(w1hi, KP),
            "w1lo": pack(w1lo, KP),
            "w2hi": pack(w2hi, KP),
            "w2lo": pack(w2lo, KP),
            "w3hi": pack(w3hi, FPAIR),
            "w3lo": pack(w3lo, FPAIR),
        })

    res = run_bass_kernel_spmd(nc, in_maps, list(range(E)))
    LAST_RESULTS = res

    # ---- combine: scatter outputs back to token order ----
    y = np.empty((T, D), dtype=np.float32)
    for e in range(E):
        toks = tok_lists[e]
        if len(toks):
            yT = res.results[e]["yt"].reshape(D, C)
            y[toks] = yT[:, :len(toks)].T.astype(np.float32)
    return y.reshape(B, S, d)


# revision 8
# speedup vs baseline: 1.2620x; 1.0414x over previous
"""MoE top-1 routing kernel for Trainium2 (8 NeuronCores, expert-parallel).

Strategy:
  - Gate (x @ Wg.T + bg, argmax) on host in float64 (exact vs fp32 ref:
    min top-2 gap ~1.2e-5 >> fp32 rounding noise).
  - Tokens grouped by expert on host (the all-to-all dispatch); core e runs
    the dense SwiGLU FFN for expert e's tokens (capacity-padded to C).
  - Combine on host (top-1 => weight 1.0); outputs come back bf16 and are
    upcast to f32.

Device kernel: all matmuls are fp8(e4m3) DoubleRow (two 128-contraction
k-tiles per matmul, 0.5 PE cycles/output-row — 2x the bf16/f32r rate).
Plain fp8 would cost ~5% accuracy, so operands are hi/lo error-compensated:

  A ~= fp8(s*A) [hi] + fp8(16*(s*A - hi))/16 [lo]   (residual ~0.16%)

and each 128-contraction product uses 3 DoubleRow terms:
  W*x  ~= Whi*xhi + Whi*xlo_s + Wlo*xhi_s    (stage A, h1 and h2)
  W3*g ~= W3hi*ghi + W3hi*glo + W3lo_s*ghi   (stage B)
with *_s = fp8(arr/16) (exact exponent shift, host-precomputed), and on
device ghi = fp8(p), glo = fp8(p - ghi) from the f32 product p = sil*ps2.
End-to-end rel-l2 vs the f32 reference: ~2.5e-3 (gate is 2e-2).

Scales (powers of 2, exact): x*8, W1*128, W2*4, W3*128, so ps1 = 1024*h1
(silu applied with input-scale 2^-10), ps2 = 32*h2, p = 32*g (|p|max ~160
< fp8 max 240), stage-B accum = 4096*y (rescaled to bf16 on copy-out).

PE cost: 54 cycles/token (36 stage A + 18 stage B) vs 72 for bf16/f32r.
Chunk c's stage-B halves are emitted between chunk c+1's stage-A pairs so
the PE doesn't wait for the ACT/DVE epilogue chain (silu -> p -> ghi/glo);
chunk 0 is emitted term-class-outer to match the DMA preload arrival order.
"""

import sys
from contextlib import ExitStack

if "/opt/trn_rl_repo" not in sys.path:
    sys.path.insert(0, "/opt/trn_rl_repo")

import numpy as np

P = 128
D = 768          # model dim
E = 8            # experts == cores
F = 469          # ffn hidden
FP = 512         # F padded to a multiple of 128
KP = 3           # k-PAIRS over D (6 k-tiles of 128)
FPAIR = 2        # f-pairs for stage-B contraction
DT = D // P      # 6 out-tiles over D
MIN_C = 128      # capacity floor
CHUNK = 512      # tokens per chunk (PSUM bank = 512 f32)

SX = 8.0         # x quant scale (2^3)
SW1 = 128.0      # W1 quant scale (2^7)
SW2 = 4.0        # W2 quant scale (2^2): p = 32*g stays under fp8 max 240
SW3 = 128.0      # W3 quant scale (2^7)
SILU_SCALE = 1.0 / (SX * SW1)        # 2^-10
OUT_SCALE = 1.0 / (SW3 * SX * SW2)   # 2^-12

_cache = {}


def _chunks(C):
    """[512]*k + small even tail (small tail keeps the pipeline drain short)."""
    sizes = []
    left = C
    while left > CHUNK:
        sizes.append(CHUNK)
        left -= CHUNK
    sizes.append(left)
    assert all(s % 2 == 0 for s in sizes) and sum(sizes) == C
    out = []
    off = 0
    for s in sizes:
        out.append((off, s))
        off += s
    return out


def _build(C):
    import concourse.bacc as bacc
    import concourse.tile as tile
    from concourse import mybir

    f32 = mybir.dt.float32
    f8 = mybir.dt.float8e4
    bf16 = mybir.dt.bfloat16
    DR = mybir.MatmulPerfMode.DoubleRow
    silu = mybir.ActivationFunctionType.Silu

    nc = bacc.Bacc("TRN2", target_bir_lowering=False, debug=False, num_devices=E)

    # DRAM inputs, host-packed fp8.
    # xq: variant axis v = (xhi, xlo_s, xhi_s); w12: v = (w1hi, w1lo, w2hi, w2lo);
    # w3: v = (w3hi, w3lo_s). Inner layout [P, 2, cols] = (partition, DR k-slot, col).
    xq_d = nc.dram_tensor("xq", [KP, P, 3, 2, C], f8, kind="ExternalInput").ap()
    w12_d = nc.dram_tensor("w12", [KP, P, 4, 2, FP], f8, kind="ExternalInput").ap()
    w3_d = nc.dram_tensor("w3", [FPAIR, P, 2, 2, D], f8, kind="ExternalInput").ap()
    yt_d = nc.dram_tensor("yt", [DT, P, C], bf16, kind="ExternalOutput").ap()

    chunks = _chunks(C)
    nch = len(chunks)

    with tile.TileContext(nc) as tc, ExitStack() as ctx:
        wpool = ctx.enter_context(tc.tile_pool(name="w", bufs=1))
        spool = ctx.enter_context(tc.tile_pool(name="s", bufs=3))
        ppool = ctx.enter_context(tc.tile_pool(name="p", bufs=3))
        gpool = ctx.enter_context(tc.tile_pool(name="g", bufs=2))
        opool = ctx.enter_context(tc.tile_pool(name="o", bufs=4))
        pspool = ctx.enter_context(tc.tile_pool(name="ps", bufs=8, space="PSUM"))

        xq = [wpool.tile([P, 3, 2, C], f8, tag=f"xq{k}", name=f"xq{k}")
              for k in range(KP)]
        w12 = [wpool.tile([P, 4, 2, FP], f8, tag=f"w12{k}", name=f"w12{k}")
               for k in range(KP)]
        w3 = [wpool.tile([P, 2, 2, D], f8, tag=f"w3{k}", name=f"w3{k}")
              for k in range(FPAIR)]

        def xdma(ci):
            n0, nn = chunks[ci]
            for kp in range(KP):
                nc.sync.dma_start(
                    xq[kp][:, :, :, n0:n0 + nn],
                    xq_d[kp, :, :, :, n0:n0 + nn],
                )

        # ---- preload, ordered to match chunk-0 consumption order ----
        n0_0, nn_0 = chunks[0]
        for kp in range(KP):
            nc.sync.dma_start(w12[kp][:, 0:2], w12_d[kp, :, 0:2])
            nc.sync.dma_start(
                xq[kp][:, :, :, n0_0:nn_0], xq_d[kp, :, :, :, n0_0:nn_0]
            )
        for kp in range(KP):
            nc.sync.dma_start(w12[kp][:, 2:4], w12_d[kp, :, 2:4])
        for fp2 in range(FPAIR):
            nc.sync.dma_start(w3[fp2][:], w3_d[fp2])
        for ci in range(1, nch):
            xdma(ci)

        def msl(m):
            return slice(m * P, (m + 1) * P)

        def stage_a_chunk0():
            """All 8 accumulators, term-class-outer (matches DMA arrival)."""
            n0, nn = chunks[0]
            csl = slice(n0, n0 + nn)
            ps1 = {m: pspool.tile([P, nn], f32, tag="ps", name=f"ps1_0_{m}")
                   for m in range(4)}
            ps2 = {m: pspool.tile([P, nn], f32, tag="ps", name=f"ps2_0_{m}")
                   for m in range(4)}
            for cls, xv in enumerate((0, 1, 2)):  # hi, lo_s, hi_s
                for psd, wv in ((ps1, 0 if cls < 2 else 1), (ps2, 2 if cls < 2 else 3)):
                    for kp in range(KP):
                        for m in range(4):
                            nc.tensor.matmul(
                                psd[m][:], w12[kp][:, wv, :, msl(m)],
                                xq[kp][:, xv, :, csl],
                                start=(cls == 0 and kp == 0),
                                stop=(cls == 2 and kp == KP - 1), perf_mode=DR,
                            )
            return ps1, ps2

        def stage_a_pair(ci, mp):
            """ps1/ps2 accumulators for f-tiles (2mp, 2mp+1); 9 DR matmuls each."""
            n0, nn = chunks[ci]
            csl = slice(n0, n0 + nn)
            ms = (2 * mp, 2 * mp + 1)
            ps1 = {m: pspool.tile([P, nn], f32, tag="ps", name=f"ps1_{ci}_{m}")
                   for m in ms}
            ps2 = {m: pspool.tile([P, nn], f32, tag="ps", name=f"ps2_{ci}_{m}")
                   for m in ms}
            for psd, wh, wl in ((ps1, 0, 1), (ps2, 2, 3)):
                for xv, wv, first, last in ((0, wh, True, False),
                                            (1, wh, False, False),
                                            (2, wl, False, True)):
                    for kp in range(KP):
                        for m in ms:
                            nc.tensor.matmul(
                                psd[m][:], w12[kp][:, wv, :, msl(m)],
                                xq[kp][:, xv, :, csl],
                                start=(first and kp == 0),
                                stop=(last and kp == KP - 1), perf_mode=DR,
                            )
            return ps1, ps2

        def epilogue_pair(ci, mp, ps1, ps2, g):
            """silu -> p -> {ghi, glo} for the two f-tiles of pair mp."""
            n0, nn = chunks[ci]
            gh, gl = g[mp]
            for j, m in enumerate((2 * mp, 2 * mp + 1)):
                sil = spool.tile([P, nn], f32, tag="sil", name=f"sil{ci}_{m}")
                nc.scalar.activation(sil[:], ps1[m][:], silu, scale=SILU_SCALE)
                p = ppool.tile([P, nn], f32, tag="p", name=f"p{ci}_{m}")
                nc.vector.tensor_mul(p[:], sil[:], ps2[m][:])
                nc.scalar.copy(gh[:, j, :], p[:])                     # ghi = fp8(p)
                nc.vector.tensor_sub(gl[:, j, :], p[:], gh[:, j, :])  # glo

        def stage_b_half(ci, half, g, ots):
            """d-tiles [3*half, 3*half+3): 6 DR matmuls each + scaled copy + store."""
            n0, nn = chunks[ci]
            for d in range(3 * half, 3 * half + 3):
                pso = pspool.tile([P, nn], f32, tag="ps", name=f"pso{ci}_{d}")
                for fp2 in range(FPAIR):
                    gh, gl = g[fp2]
                    nc.tensor.matmul(pso[:], w3[fp2][:, 0, :, msl(d)], gh[:],
                                     start=(fp2 == 0), stop=False, perf_mode=DR)
                    nc.tensor.matmul(pso[:], w3[fp2][:, 0, :, msl(d)], gl[:],
                                     start=False, stop=False, perf_mode=DR)
                    nc.tensor.matmul(pso[:], w3[fp2][:, 1, :, msl(d)], gh[:],
                                     start=False, stop=(fp2 == FPAIR - 1),
                                     perf_mode=DR)
                q, r = divmod(d, 2)
                if r == 0:
                    ot = opool.tile([P, 2, nn], bf16, tag="ot", name=f"ot{ci}_{q}")
                    ots[q] = ot
                    nc.scalar.mul(ot[:, 0, :], pso[:], OUT_SCALE)
                else:
                    ot = ots[q]  # slot 0 filled by d-1 (possibly in half 0)
                    nc.vector.tensor_scalar_mul(ot[:, 1, :], pso[:], OUT_SCALE)
                    nc.sync.dma_start(
                        yt_d[2 * q:2 * q + 2, :, n0:n0 + nn].rearrange("j p c -> p j c"),
                        ot[:],
                    )

        def gtiles(ci):
            nn = chunks[ci][1]
            return {mp: (gpool.tile([P, 2, nn], f8, tag=f"gh{mp}", name=f"gh{ci}_{mp}"),
                         gpool.tile([P, 2, nn], f8, tag=f"gl{mp}", name=f"gl{ci}_{mp}"))
                    for mp in range(2)}

        # ---- software-pipelined emission ----
        g0 = gtiles(0)
        ps1_0, ps2_0 = stage_a_chunk0()
        epilogue_pair(0, 0, ps1_0, ps2_0, g0)
        epilogue_pair(0, 1, ps1_0, ps2_0, g0)
        prev = (0, g0, {})
        for ci in range(1, nch):
            g = gtiles(ci)
            ps1a, ps2a = stage_a_pair(ci, 0)
            stage_b_half(prev[0], 0, prev[1], prev[2])
            epilogue_pair(ci, 0, ps1a, ps2a, g)
            ps1b, ps2b = stage_a_pair(ci, 1)
            stage_b_half(prev[0], 1, prev[1], prev[2])
            epilogue_pair(ci, 1, ps1b, ps2b, g)
            prev = (ci, g, {})
        stage_b_half(prev[0], 0, prev[1], prev[2])
        stage_b_half(prev[0], 1, prev[1], prev[2])

    nc.compile()
    return nc


LAST_RESULTS = None  # BassKernelResults of the most recent run (for test harness)


def kernel(x, Wg, bg, W1, W2, W3):
    global LAST_RESULTS
    import ml_dtypes
    from concourse.bass_utils import run_bass_kernel_spmd

    f8np = ml_dtypes.float8_e4m3

    x = np.asarray(x)
    Wg, bg = np.asarray(Wg), np.asarray(bg)
    W1, W2, W3 = np.asarray(W1), np.asarray(W2), np.asarray(W3)
    B, S, d = x.shape
    T = B * S
    assert d == D and Wg.shape == (E, D)

    xf = np.ascontiguousarray(x.reshape(T, D))

    # ---- host gate + top-1 routing (fp64: exact vs any fp32 backend) ----
    gate = xf.astype(np.float64) @ Wg.astype(np.float64).T + bg.astype(np.float64)
    eid = np.argmax(gate, axis=1)
    counts = np.bincount(eid, minlength=E)
    order = np.argsort(eid, kind="stable")
    offs = np.concatenate(([0], np.cumsum(counts)))

    C = max(MIN_C, 2 * int(-(-counts.max() // 2)))
    if C not in _cache:
        _cache[C] = _build(C)
    nc = _cache[C]

    def q8(a):
        return a.astype(f8np)

    def hi_lo(a, s):
        hi = q8(a * s)
        lo = q8((a * s - hi.astype(np.float32)) * 16.0)
        return hi, lo

    def pack(a, npair):
        # [R, cols] with R = npair*2*128 -> [npair, P, 2, cols]
        cols = a.shape[1]
        return a.reshape(npair, 2, P, cols).transpose(0, 2, 1, 3)

    # ---- per-core inputs (dispatch) ----
    in_maps = []
    tok_lists = []
    for e in range(E):
        toks = order[offs[e]:offs[e + 1]]
        tok_lists.append(toks)
        ce = len(toks)

        xT = np.zeros((D, C), dtype=np.float32)
        if ce:
            xT[:, :ce] = xf[toks].T
        xhi, xlo = hi_lo(xT, SX)
        xlos = q8(xlo.astype(np.float32) / 16.0)
        xhis = q8(xhi.astype(np.float32) / 16.0)
        xq = np.stack([pack(v, KP) for v in (xhi, xlos, xhis)], axis=2)

        w1T = np.zeros((D, FP), dtype=np.float32)
        w1T[:, :F] = W1[e].T
        w1hi, w1lo = hi_lo(w1T, SW1)
        w2T = np.zeros((D, FP), dtype=np.float32)
        w2T[:, :F] = W2[e].T
        w2hi, w2lo = hi_lo(w2T, SW2)
        w12 = np.stack([pack(v, KP) for v in (w1hi, w1lo, w2hi, w2lo)], axis=2)

        w3T = np.zeros((FP, D), dtype=np.float32)
        w3T[:F, :] = W3[e].T
        w3hi, w3lo = hi_lo(w3T, SW3)
        w3ls = q8(w3lo.astype(np.float32) / 16.0)
        w3 = np.stack([pack(v, FPAIR) for v in (w3hi, w3ls)], axis=2)

        in_maps.append({
            "xq": np.ascontiguousarray(xq),
            "w12": np.ascontiguousarray(w12),
            "w3": np.ascontiguousarray(w3),
        })

    res = run_bass_kernel_spmd(nc, in_maps, list(range(E)))
    LAST_RESULTS = res

    # ---- combine: scatter outputs back to token order ----
    y = np.empty((T, D), dtype=np.float32)
    for e in range(E):
        toks = tok_lists[e]
        if len(toks):
            yT = res.results[e]["yt"].reshape(D, C)
            y[toks] = yT[:, :len(toks)].T.astype(np.float32)
    return y.reshape(B, S, d)
